# revision 1
# baseline (speedup 1.0000x reference)
# Trainium2 Bass kernel for the KerasLMU problem.
#
# Math: per time step t (T=1024 steps),
#   u_t = x_t @ e_x                       (B,1)
#   m_t = m_{t-1} @ A.T + b_row * u_t     (B,256)   -- linear recurrence
#   h_t = lrelu(x_t @ W_x + h_{t-1} @ W_h.T + m_t @ W_m.T)
#
# Reformulation used here:
#   m_t = sum_{k=0..t-1?} A^k b u_{t-k}  (causal convolution), so
#   c_t := x_t @ W_x + m_t @ W_m.T = x_t @ W_x + sum_k G[k] u_{t-k}
#   with G[k] = W_m @ (A^k b) precomputed host-side in float64 (exact
#   function of the constant inputs A, Bv, W_m).
# The only sequential device work left is h_t = lrelu(c_t + h_{t-1} @ W_h.T),
# run as a 1024-step loop of 16 bf16 [128x128]x[128,8] matmuls + a 2-op
# DVE/ACT epilogue per step, with h kept transposed ([hidden, batch]) so the
# epilogue runs on full 128-partition tiles.
#
# Sharding: data-parallel over batch. 64 batch rows -> 8 cores x 8 rows.
# All weights replicated; no collectives.

import os
import sys

sys.path.insert(0, "/opt/trn_rl_repo")

import numpy as np
import ml_dtypes

import concourse.bass as bass
import concourse.tile as tile
from concourse import bacc, mybir
from concourse.bass_utils import run_bass_kernel_spmd

F32 = mybir.dt.float32
BF16 = mybir.dt.bfloat16
BF = ml_dtypes.bfloat16

NCORES = 8
BATCH = 64
BC = BATCH // NCORES          # batch rows per core = 8
FEAT = 128
HID = 512
ORDER = 256
TFULL = 1024
TBLK = 64                     # seq-loop steps per DMA block

# module-level stash for test harness introspection
last_run_info = {}


def _dap(handle, offset, dims):
    """Build an explicit AP on a DRAM tensor: dims = [[step, count], ...]
    (element units; first dim pairs with the SBUF partition dim)."""
    base = handle[:]
    return bass.AP(tensor=base.tensor, offset=offset, ap=[list(d) for d in dims])


def build_nc(T=TFULL, tblk=TBLK, debug=False):
    """Emit the per-core Bass/Tile program (SPMD; all cores identical)."""
    assert T % 1024 == 0 or T in (128, 256, 512), T
    nblk = T // tblk
    BT = BC * T                       # rows of x per core
    nxt = BT // 128                   # 128-row x tiles
    th_n = T // 512 if T >= 512 else 1  # 512-wide tau halves in conv
    tw = min(T, 512)                  # conv tau tile width

    nc = bacc.Bacc(None, target_bir_lowering=False)
    x_d = nc.declare_dram_parameter("x", [BT, FEAT], F32, isOutput=False)
    whT_d = nc.declare_dram_parameter("whT", [HID, HID], BF16, isOutput=False)
    g_d = nc.declare_dram_parameter("g", [T, HID], F32, isOutput=False)
    wx_d = nc.declare_dram_parameter("wx", [FEAT, HID], F32, isOutput=False)
    ex_d = nc.declare_dram_parameter("ex", [FEAT, 1], F32, isOutput=False)
    id_d = nc.declare_dram_parameter("ident", [128, 128], F32, isOutput=False)
    out_d = nc.declare_dram_parameter("out", [BT, HID], BF16, isOutput=True)

    UPADW = 512 + T                   # zeros(512) ++ u(T)
    upad_d = nc.dram_tensor("u_pad", [BC, UPADW], F32)
    cT_d = nc.dram_tensor("cT", [BC, 4, 128, T], F32)   # [b][jt][p][tau]
    if debug:
        dbg_u = nc.declare_dram_parameter("dbg_u", [BC, UPADW], F32,
                                          isOutput=True)
        dbg_c = nc.declare_dram_parameter("dbg_c", [BC, 4, 128, T], F32,
                                          isOutput=True)

    USHW = T + 384                    # Qi domain width
    KCN = T // 128                    # lag chunks

    with tile.TileContext(nc) as tc:
        with (
            tc.tile_pool(name="consts", bufs=1) as consts,
            tc.tile_pool(name="work", bufs=4) as work,
            tc.tile_pool(name="cstage", bufs=4) as cstage,
            tc.tile_pool(name="cblk", bufs=2) as cblk,
            tc.tile_pool(name="hout", bufs=2) as hpool,
            tc.tile_pool(name="psA", bufs=4, space="PSUM") as psA,
            tc.tile_pool(name="psS", bufs=4, space="PSUM") as psS,
        ):
            # ---- resident constants -------------------------------------
            whT_sb = consts.tile([128, 4, HID], BF16)
            for kc in range(4):
                nc.sync.dma_start(out=whT_sb[:, kc, :],
                                  in_=whT_d[kc * 128:(kc + 1) * 128, :])
            g_sb = consts.tile([128, KCN, HID], F32)
            for kc in range(KCN):
                nc.sync.dma_start(out=g_sb[:, kc, :],
                                  in_=g_d[kc * 128:(kc + 1) * 128, :])
            wx_sb = consts.tile([128, HID], F32)
            nc.sync.dma_start(out=wx_sb, in_=wx_d[:, :])
            ex_sb = consts.tile([128, 1], F32)
            nc.sync.dma_start(out=ex_sb, in_=ex_d[:, :])
            id_sb = consts.tile([128, 128], F32)
            nc.sync.dma_start(out=id_sb, in_=id_d[:, :])

            xT_sb = consts.tile([128, BT], F32)     # x.T : [feat, (b,tau)]
            ushr = consts.tile([128, BC, USHW], F32)  # reversed u shifts
            zrow = consts.tile([1, 512], F32)
            nc.vector.memset(zrow, 0.0)
            h0 = consts.tile([128, 4, BC], BF16)
            nc.vector.memset(h0, 0.0)

            # ---- phase A: x transpose (PE) ------------------------------
            for r in range(nxt):
                x_tile = work.tile([128, 128], F32, tag="xt")
                nc.sync.dma_start(out=x_tile,
                                  in_=x_d[r * 128:(r + 1) * 128, :])
                ps = psA.tile([128, 128], F32, tag="ps")
                nc.tensor.transpose(ps, x_tile, id_sb)
                dst = xT_sb[:, r * 128:(r + 1) * 128]
                if r % 2 == 0:
                    nc.scalar.copy(dst, ps)
                else:
                    nc.vector.tensor_copy(dst, ps)

            # ---- phase B: u = x @ e_x  ->  u_pad DRAM -------------------
            for b8 in range(BC):
                urow = work.tile([1, UPADW], F32, tag="urow")
                nc.vector.tensor_copy(urow[:, 0:512], zrow)
                for th in range((T + 511) // 512):
                    w = min(512, T - th * 512)
                    ps = psA.tile([1, 512], F32, tag="ps")
                    nc.tensor.matmul(ps[:, :w], lhsT=ex_sb,
                                     rhs=xT_sb[:, b8 * T + th * 512:
                                               b8 * T + th * 512 + w],
                                     start=True, stop=True)
                    nc.scalar.copy(urow[:, 512 + th * 512:512 + th * 512 + w],
                                   ps[:, :w])
                nc.gpsimd.dma_start(out=upad_d[b8:b8 + 1, :], in_=urow)

            # ---- phase C: build reversed shift matrix -------------------
            # ushr[p, b, Qi] = u_pad[b][1 + Qi + p]
            for b8 in range(BC):
                nc.gpsimd.dma_start(
                    out=ushr[:, b8, :],
                    in_=_dap(upad_d, b8 * UPADW + 1, [[1, 128], [1, USHW]]))

            # ---- phase D: c.T = conv(G, u) + W_x.T @ x.T  -> cT DRAM ----
            ev = 0
            for b8 in range(BC):
                for jt in range(4):
                    for th in range(th_n):
                        ps = psA.tile([128, tw], F32, tag="ps")
                        first = True
                        kmax = min(KCN, 4 * th + tw // 128)
                        for kc in range(kmax):
                            qi0 = 384 + 512 * th - 128 * kc
                            nc.tensor.matmul(
                                ps, lhsT=g_sb[:, kc, jt * 128:(jt + 1) * 128],
                                rhs=ushr[:, b8, qi0:qi0 + tw],
                                start=first, stop=False)
                            first = False
                        nc.tensor.matmul(
                            ps, lhsT=wx_sb[:, jt * 128:(jt + 1) * 128],
                            rhs=xT_sb[:, b8 * T + th * 512:b8 * T + th * 512 + tw],
                            start=False, stop=True)
                        cs = cstage.tile([128, tw], F32, tag="cs")
                        if ev % 2 == 0:
                            nc.scalar.copy(cs, ps)
                        else:
                            nc.vector.tensor_copy(cs, ps)
                        ev += 1
                        nc.sync.dma_start(
                            out=cT_d[b8, jt, :, th * 512:th * 512 + tw],
                            in_=cs)

            if debug:
                nc.sync.dma_start(out=dbg_u[:, :], in_=upad_d[:, :])
                nc.sync.dma_start(out=dbg_c[:, :, :, :], in_=cT_d[:, :, :, :])

            # ---- phase E: sequential h recurrence -----------------------
            # Warm all psS banks once: a start=True pass clears the
            # pending-zero bits over our [128, 4*BC] region so the per-step
            # matmuls can run start=False and accumulate onto a DVE-prewritten
            # c_t (keeps the c add off the PE critical path).
            warm = [psS.tile([128, 4, BC], F32, tag="pss", name=f"warm{i}")
                    for i in range(4)]
            for mc in range(4):
                for wt in warm:
                    nc.tensor.matmul(
                        wt[:, mc, :],
                        lhsT=whT_sb[:, 0, mc * 128:(mc + 1) * 128],
                        rhs=h0[:, 0, :],
                        start=(mc == 0), stop=(mc == 3),
                        skip_group_check=True)

            h_prev = h0                      # [128, 4(kc), BC] bf16
            h_prev_dt = None
            ps_cur = None
            for blk in range(nblk):
                t0 = blk * tblk
                cb = cblk.tile([128, 4, BC, tblk], F32, tag="cb")
                for jt in range(4):
                    nc.sync.dma_start(
                        out=cb[:, jt, :, :],
                        in_=_dap(cT_d, jt * 128 * T + t0,
                                 [[T, 128], [4 * 128 * T, BC], [1, tblk]]))
                hb = hpool.tile([128, tblk, 4, BC], BF16, tag="hb")
                if ps_cur is None:
                    ps_cur = psS.tile([128, 4, BC], F32, tag="pss")
                    nc.vector.tensor_copy(ps_cur, cb[:, :, :, 0])
                for dt in range(tblk):
                    ps = ps_cur
                    # prefetch next step's c into its psum bank (DVE, off
                    # the PE critical path)
                    if dt + 1 < tblk:
                        ps_cur = psS.tile([128, 4, BC], F32, tag="pss")
                        nc.vector.tensor_copy(ps_cur, cb[:, :, :, dt + 1])
                    else:
                        ps_cur = None
                    for kc in range(4):
                        rhs = (h_prev[:, kc, :] if h_prev_dt is None
                               else h_prev[:, h_prev_dt, kc, :])
                        for mc in range(4):
                            nc.tensor.matmul(
                                ps[:, mc, :],
                                lhsT=whT_sb[:, kc, mc * 128:(mc + 1) * 128],
                                rhs=rhs,
                                start=False, stop=False,
                                skip_group_check=True)
                    for half in range(2):
                        nc.scalar.activation(
                            hb[:, dt, 2 * half:2 * half + 2, :],
                            ps[:, 2 * half:2 * half + 2, :],
                            mybir.ActivationFunctionType.Prelu,
                            alpha=0.2)
                    h_prev = hb
                    h_prev_dt = dt
                # write block to DRAM out: row r=(b*T+t0+dt), col=128*mc+p
                for b8 in range(BC):
                    nc.sync.dma_start(
                        out=_dap(out_d, (b8 * T + t0) * HID,
                                 [[1, 128], [HID, tblk], [128, 4]]),
                        in_=hb[:, :, :, b8])
    nc.compile()
    return nc


_nc_cache = {}


def _get_nc(T, tblk):
    key = (T, tblk)
    if key not in _nc_cache:
        _nc_cache[key] = build_nc(T, tblk)
    return _nc_cache[key]


def host_prep(x, A, Bv, W_x, e_x, W_h, W_m, T):
    """Host-side constant prep (float64, exact fn of constant inputs)."""
    order = A.shape[0]
    A64 = A.astype(np.float64)
    b64 = Bv[:, 0].astype(np.float64)
    Hk = np.empty((T, order))
    v = b64.copy()
    for k in range(T):
        Hk[k] = v
        v = A64 @ v
    G = (Hk @ W_m.T.astype(np.float64)).astype(np.float32)      # (T, 512)
    # reverse lag index within each 128-chunk (matches reversed u-shift rows)
    Gr = G.reshape(T // 128, 128, -1)[:, ::-1, :].reshape(T, -1).copy()
    whT = np.ascontiguousarray(W_h.T).astype(BF)
    return Gr, whT


def kernel(x, A, Bv, W_x, e_x, W_h, W_m, T=TFULL, tblk=TBLK):
    x = np.asarray(x, np.float32)
    A = np.asarray(A, np.float32)
    Bv = np.asarray(Bv, np.float32)
    W_x = np.asarray(W_x, np.float32)
    e_x = np.asarray(e_x, np.float32)
    W_h = np.asarray(W_h, np.float32)
    W_m = np.asarray(W_m, np.float32)

    Gr, whT = host_prep(x, A, Bv, W_x, e_x, W_h, W_m, T)
    ident = np.eye(128, dtype=np.float32)

    nc = _get_nc(T, tblk)
    B = x.shape[0]
    in_maps = []
    for c in range(NCORES):
        xs = np.ascontiguousarray(
            x[c * BC:(c + 1) * BC, 1:T + 1, :].reshape(BC * T, FEAT))
        in_maps.append({
            "x": xs, "whT": whT, "g": Gr, "wx": W_x, "ex": e_x,
            "ident": ident,
        })
    trace = bool(int(os.environ.get("KERNEL_TRACE", "0")))
    res = run_bass_kernel_spmd(nc, in_maps, list(range(NCORES)), trace=trace)
    last_run_info.clear()
    last_run_info.update(
        exec_time_ns=res.exec_time_ns,
        mean_exec_time_ns=res.mean_exec_time_ns,
        profile_json=res.profile_json,
    )
    out = np.empty((B, T, HID), np.float32)
    for c in range(NCORES):
        o = res.results[c]["out"].astype(np.float32).reshape(BC, T, HID)
        out[c * BC:(c + 1) * BC] = o
    return out



# revision 3
# speedup vs baseline: 5.1627x; 5.1627x over previous
# Trainium2 Bass kernel for the KerasLMU problem.
#
# Math: per time step t (T=1024 steps),
#   u_t = x_t @ e_x                       (B,1)
#   m_t = m_{t-1} @ A.T + b_row * u_t     (B,256)   -- linear recurrence
#   h_t = lrelu(x_t @ W_x + h_{t-1} @ W_h.T + m_t @ W_m.T)
#
# Reformulation: m_t = sum_k A^k b u_{t-k} (causal convolution), so
#   c_t := x_t @ W_x + m_t @ W_m.T = x_t @ W_x + sum_k G[k] u_{t-k}
# with G[k] = W_m @ (A^k b) precomputed host-side in float64. The only
# sequential device work left is h_t = lrelu(c_t + h_{t-1} @ W_h.T).
#
# Device pipeline (per core, data-parallel over batch; 8 rows/core):
#   A: PE-transpose x -> xT (bf16)          [feat, (b,tau)]
#   B: u = e_x^T @ xT -> u_pad DRAM (bf16, 512 zeros prefix)
#   C: overlapping-window DMA -> ushr (128 shifted copies of u)
#   D: c^T = conv(G,u) + W_x^T xT, bf16 matmuls, f32 PSUM, c kept
#      RESIDENT IN SBUF as bf16 [128, 4, BC, T] (no DRAM round-trip)
#   E: 1024-step recurrence. Per step 16 [128x128]x[128x8] bf16
#      matmuls in a staggered 4-bank slot schedule so each PSUM bank
#      completes ~8 slots before its h-chunk is consumed next step,
#      hiding the PE->act->PE latency (~570ns). Leaky-ReLU alternates
#      between Scalar (Prelu) and Vector (scalar_tensor_tensor) so
#      neither engine serializes the chain. h blocks DMA to DRAM in
#      SBUF-native layout (contiguous 4KB descriptors); the final
#      [b, t, hid] permutation happens host-side in numpy.

import os
import sys

sys.path.insert(0, "/opt/trn_rl_repo")

import numpy as np
import ml_dtypes

import concourse.bass as bass
import concourse.tile as tile
from concourse import bacc, mybir
from concourse.bass_utils import run_bass_kernel_spmd

F32 = mybir.dt.float32
BF16 = mybir.dt.bfloat16
BF = ml_dtypes.bfloat16

NCORES = 8
BATCH = 64
BC = BATCH // NCORES          # batch rows per core = 8
FEAT = 128
HID = 512
ORDER = 256
TFULL = 1024
TBLK = 64                     # seq-loop steps per out-DMA block

# phase-E slot schedule: (mc, kc) per PE slot. Banks complete at slots
# 9/11/14/16; each bank consumes h-chunks in last-step completion order
# (c0 earliest), max backward slack 8 slots.
SLOT2MK = [(0, 0), (1, 0), (0, 1), (1, 1), (2, 0), (0, 2), (1, 2), (2, 1),
           (0, 3), (3, 0), (1, 3), (2, 2), (3, 1), (2, 3), (3, 2), (3, 3)]
assert sorted(SLOT2MK) == [(m, k) for m in range(4) for k in range(4)]

# module-level stash for test harness introspection
last_run_info = {}


def _dap(handle, offset, dims):
    """Build an explicit AP on a DRAM tensor: dims = [[step, count], ...]
    (element units; first dim pairs with the SBUF partition dim)."""
    base = handle[:]
    return bass.AP(tensor=base.tensor, offset=offset, ap=[list(d) for d in dims])


def build_nc(T=TFULL, tblk=TBLK):
    """Emit the per-core Bass/Tile program (SPMD; all cores identical)."""
    nblk = T // tblk
    BT = BC * T                       # rows of x per core
    nxt = BT // 128                   # 128-row x tiles
    th_n = T // 512 if T >= 512 else 1  # 512-wide tau halves in conv
    tw = min(T, 512)                  # conv tau tile width
    OWB = tblk * 4 * BC               # out elements per partition per block

    nc = bacc.Bacc(None, target_bir_lowering=False)
    x_d = nc.declare_dram_parameter("x", [BT, FEAT], F32, isOutput=False)
    whT_d = nc.declare_dram_parameter("whT", [HID, HID], BF16, isOutput=False)
    g_d = nc.declare_dram_parameter("g", [T, HID], BF16, isOutput=False)
    wx_d = nc.declare_dram_parameter("wx", [FEAT, HID], BF16, isOutput=False)
    ex_d = nc.declare_dram_parameter("ex", [FEAT, 1], BF16, isOutput=False)
    id_d = nc.declare_dram_parameter("ident", [128, 128], F32, isOutput=False)
    # out in SBUF-native block layout: row = blk*128 + p, col = dt*32+mc*8+b8
    out_d = nc.declare_dram_parameter("out", [nblk * 128, OWB], BF16,
                                      isOutput=True)

    UPADW = 512 + T                   # zeros(512) ++ u(T)
    upad_d = nc.dram_tensor("u_pad", [BC, UPADW], BF16)

    USHW = T + 384                    # Qi domain width
    KCN = T // 128                    # lag chunks

    with tile.TileContext(nc) as tc:
        with (
            tc.tile_pool(name="consts", bufs=1) as consts,
            tc.tile_pool(name="work", bufs=4) as work,
            tc.tile_pool(name="hout", bufs=2) as hpool,
            tc.tile_pool(name="psA", bufs=4, space="PSUM") as psA,
            tc.tile_pool(name="psS", bufs=4, space="PSUM") as psS,
        ):
            # ---- resident constants -------------------------------------
            whT_sb = consts.tile([128, 4, HID], BF16)
            for kc in range(4):
                nc.sync.dma_start(out=whT_sb[:, kc, :],
                                  in_=whT_d[kc * 128:(kc + 1) * 128, :])
            g_sb = consts.tile([128, KCN, HID], BF16)
            for kc in range(KCN):
                nc.sync.dma_start(out=g_sb[:, kc, :],
                                  in_=g_d[kc * 128:(kc + 1) * 128, :])
            wx_sb = consts.tile([128, HID], BF16)
            nc.sync.dma_start(out=wx_sb, in_=wx_d[:, :])
            ex_sb = consts.tile([128, 1], BF16)
            nc.sync.dma_start(out=ex_sb, in_=ex_d[:, :])
            id_sb = consts.tile([128, 128], F32)
            nc.sync.dma_start(out=id_sb, in_=id_d[:, :])

            xT_sb = consts.tile([128, BT], BF16)    # x.T : [feat, (b,tau)]
            ushr = consts.tile([128, BC, USHW], BF16)  # reversed u shifts
            c_sb = consts.tile([128, 4, BC, T], BF16)  # c^T resident
            zrow = consts.tile([1, 512], BF16)
            nc.vector.memset(zrow, 0.0)
            h0 = consts.tile([128, 4, BC], BF16)
            nc.vector.memset(h0, 0.0)

            # ---- phase A: x transpose (PE) ------------------------------
            for r in range(nxt):
                x_tile = work.tile([128, 128], F32, tag="xt")
                nc.sync.dma_start(out=x_tile,
                                  in_=x_d[r * 128:(r + 1) * 128, :])
                ps = psA.tile([128, 128], F32, tag="ps")
                nc.tensor.transpose(ps, x_tile, id_sb)
                dst = xT_sb[:, r * 128:(r + 1) * 128]
                if r % 2 == 0:
                    nc.scalar.copy(dst, ps)
                else:
                    nc.vector.tensor_copy(dst, ps)

            # ---- phase B: u = x @ e_x  ->  u_pad DRAM -------------------
            for b8 in range(BC):
                urow = work.tile([1, UPADW], BF16, tag="urow")
                nc.vector.tensor_copy(urow[:, 0:512], zrow)
                for th in range((T + 511) // 512):
                    w = min(512, T - th * 512)
                    ps = psA.tile([1, 512], F32, tag="ps")
                    nc.tensor.matmul(ps[:, :w], lhsT=ex_sb,
                                     rhs=xT_sb[:, b8 * T + th * 512:
                                               b8 * T + th * 512 + w],
                                     start=True, stop=True)
                    nc.scalar.copy(urow[:, 512 + th * 512:512 + th * 512 + w],
                                   ps[:, :w])
                nc.gpsimd.dma_start(out=upad_d[b8:b8 + 1, :], in_=urow)

            # ---- phase C: build reversed shift matrix -------------------
            # ushr[p, b, Qi] = u_pad[b][1 + Qi + p]
            for b8 in range(BC):
                nc.gpsimd.dma_start(
                    out=ushr[:, b8, :],
                    in_=_dap(upad_d, b8 * UPADW + 1, [[1, 128], [1, USHW]]))

            # ---- phase D: c^T = conv(G, u) + W_x^T @ x^T -> c_sb (SBUF) -
            ev = 0
            for b8 in range(BC):
                for jt in range(4):
                    for th in range(th_n):
                        ps = psA.tile([128, tw], F32, tag="ps")
                        first = True
                        kmax = min(KCN, 4 * th + tw // 128)
                        for kc in range(kmax):
                            qi0 = 384 + 512 * th - 128 * kc
                            nc.tensor.matmul(
                                ps, lhsT=g_sb[:, kc, jt * 128:(jt + 1) * 128],
                                rhs=ushr[:, b8, qi0:qi0 + tw],
                                start=first, stop=False)
                            first = False
                        nc.tensor.matmul(
                            ps, lhsT=wx_sb[:, jt * 128:(jt + 1) * 128],
                            rhs=xT_sb[:, b8 * T + th * 512:b8 * T + th * 512 + tw],
                            start=False, stop=True)
                        dst = c_sb[:, jt, b8, th * 512:th * 512 + tw]
                        if ev % 2 == 0:
                            nc.scalar.copy(dst, ps)
                        else:
                            nc.vector.tensor_copy(dst, ps)
                        ev += 1

            # ---- phase E: sequential h recurrence -----------------------
            # Warm all psS banks once: a start=True pass clears the
            # pending-zero bits over our [128, 4*BC] region so the per-step
            # matmuls can run start=False and accumulate onto a DVE/ACT
            # prewritten c_t (keeps the c add off the PE critical path).
            warm = [psS.tile([128, 4, BC], F32, tag="pss", name=f"warm{i}")
                    for i in range(4)]
            for mc in range(4):
                for wt in warm:
                    nc.tensor.matmul(
                        wt[:, mc, :],
                        lhsT=whT_sb[:, 0, mc * 128:(mc + 1) * 128],
                        rhs=h0[:, 0, :],
                        start=(mc == 0), stop=(mc == 3),
                        skip_group_check=True)

            h_prev = h0                      # [128, 4(kc), BC] bf16
            h_prev_dt = None
            ps_cur = psS.tile([128, 4, BC], F32, tag="pss")
            nc.vector.tensor_copy(ps_cur, c_sb[:, :, :, 0])
            for blk in range(nblk):
                t0 = blk * tblk
                hb = hpool.tile([128, tblk, 4, BC], BF16, tag="hb")
                for dt in range(tblk):
                    t = t0 + dt
                    ps = ps_cur
                    # prefetch next step's c into its psum bank (DVE, off
                    # the PE critical path)
                    if t + 1 < T:
                        ps_cur = psS.tile([128, 4, BC], F32, tag="pss")
                        nc.vector.tensor_copy(ps_cur, c_sb[:, :, :, t + 1])
                    ndone = [0, 0, 0, 0]
                    for mc, kc in SLOT2MK:
                        rhs = (h_prev[:, kc, :] if h_prev_dt is None
                               else h_prev[:, h_prev_dt, kc, :])
                        nc.tensor.matmul(
                            ps[:, mc, :],
                            lhsT=whT_sb[:, kc, mc * 128:(mc + 1) * 128],
                            rhs=rhs,
                            start=False, stop=False,
                            skip_group_check=True)
                        ndone[mc] += 1
                        if ndone[mc] == 4:
                            if mc != 1:
                                nc.scalar.activation(
                                    hb[:, dt, mc, :], ps[:, mc, :],
                                    mybir.ActivationFunctionType.Prelu,
                                    alpha=0.2)
                            else:
                                # DVE leaky-relu: only one PSUM input per
                                # instruction is allowed, so stage 0.2*x
                                # through SBUF then max against PSUM.
                                lrt = work.tile([128, BC], F32, tag="lrt")
                                nc.vector.tensor_scalar_mul(
                                    lrt, ps[:, mc, :], 0.2)
                                nc.vector.tensor_tensor(
                                    hb[:, dt, mc, :], lrt, ps[:, mc, :],
                                    op=mybir.AluOpType.max)
                    h_prev = hb
                    h_prev_dt = dt
                # write block: SBUF-native layout, contiguous per partition
                nc.sync.dma_start(
                    out=_dap(out_d, blk * 128 * OWB,
                             [[OWB, 128], [4 * BC, tblk], [BC, 4], [1, BC]]),
                    in_=hb)
    nc.compile()
    return nc


_nc_cache = {}


def _get_nc(T, tblk):
    key = (T, tblk)
    if key not in _nc_cache:
        _nc_cache[key] = build_nc(T, tblk)
    return _nc_cache[key]


def host_prep(x, A, Bv, W_x, e_x, W_h, W_m, T):
    """Host-side constant prep (float64, exact fn of constant inputs)."""
    order = A.shape[0]
    A64 = A.astype(np.float64)
    b64 = Bv[:, 0].astype(np.float64)
    Hk = np.empty((T, order))
    v = b64.copy()
    for k in range(T):
        Hk[k] = v
        v = A64 @ v
    G = (Hk @ W_m.T.astype(np.float64)).astype(np.float32)      # (T, 512)
    # reverse lag index within each 128-chunk (matches reversed u-shift rows)
    Gr = G.reshape(T // 128, 128, -1)[:, ::-1, :].reshape(T, -1)
    Gr = np.ascontiguousarray(Gr).astype(BF)
    whT = np.ascontiguousarray(W_h.T).astype(BF)
    return Gr, whT


def kernel(x, A, Bv, W_x, e_x, W_h, W_m, T=TFULL, tblk=TBLK):
    x = np.asarray(x, np.float32)
    A = np.asarray(A, np.float32)
    Bv = np.asarray(Bv, np.float32)
    W_x = np.asarray(W_x, np.float32)
    e_x = np.asarray(e_x, np.float32)
    W_h = np.asarray(W_h, np.float32)
    W_m = np.asarray(W_m, np.float32)

    Gr, whT = host_prep(x, A, Bv, W_x, e_x, W_h, W_m, T)
    ident = np.eye(128, dtype=np.float32)

    nc = _get_nc(T, tblk)
    B = x.shape[0]
    nblk = T // tblk
    in_maps = []
    for c in range(NCORES):
        xs = np.ascontiguousarray(
            x[c * BC:(c + 1) * BC, 1:T + 1, :].reshape(BC * T, FEAT))
        in_maps.append({
            "x": xs, "whT": whT, "g": Gr, "wx": W_x.astype(BF),
            "ex": e_x.astype(BF), "ident": ident,
        })
    trace = bool(int(os.environ.get("KERNEL_TRACE", "0")))
    res = run_bass_kernel_spmd(nc, in_maps, list(range(NCORES)), trace=trace)
    last_run_info.clear()
    last_run_info.update(
        exec_time_ns=res.exec_time_ns,
        mean_exec_time_ns=res.mean_exec_time_ns,
        profile_json=res.profile_json,
    )
    out = np.empty((B, T, HID), np.float32)
    for c in range(NCORES):
        o = np.asarray(res.results[c]["out"]).astype(np.float32)
        # [blk*128+p, dt*32+mc*8+b8] -> [b8, blk*tblk+dt, mc*128+p]
        o = o.reshape(nblk, 128, tblk, 4, BC)
        o = o.transpose(4, 0, 2, 3, 1).reshape(BC, T, HID)
        out[c * BC:(c + 1) * BC] = o
    return out


# revision 5
# speedup vs baseline: 8.8517x; 1.7146x over previous
# Trainium2 Bass kernel for the KerasLMU problem.
#
# Math: per time step t (T=1024 steps),
#   u_t = x_t @ e_x                       (B,1)
#   m_t = m_{t-1} @ A.T + b_row * u_t     (B,256)   -- linear recurrence
#   h_t = lrelu(x_t @ W_x + h_{t-1} @ W_h.T + m_t @ W_m.T)
#
# Reformulation: m_t = sum_k A^k b u_{t-k} (causal convolution), so
#   c_t := x_t @ W_x + m_t @ W_m.T = x_t @ W_x + sum_k G[k] u_{t-k}
# with G[k] = W_m @ (A^k b) precomputed host-side in float64. The only
# sequential device work left is h_t = lrelu(c_t + h_{t-1} @ W_h.T).
#
# Device pipeline (per core, data-parallel over batch; 8 rows/core):
#   A: PE-transpose x -> xT (bf16)          [feat, (b,tau)]
#   B: u = e_x^T @ xT -> u_pad DRAM (bf16, 512 zeros prefix)
#   C: overlapping-window DMA -> ushr (128 shifted copies of u)
#   D: c^T = conv(G,u) + W_x^T xT, bf16 matmuls, f32 PSUM, c kept
#      RESIDENT IN SBUF as bf16 [128, 4, BC, T] (no DRAM round-trip)
#   E: 1024-step recurrence. Per step 16 [128x128]x[128x8] bf16
#      matmuls in a staggered 4-bank slot schedule so each PSUM bank
#      completes ~8 slots before its h-chunk is consumed next step,
#      hiding the PE->act->PE latency (~570ns). Leaky-ReLU alternates
#      between Scalar (Prelu) and Vector (scalar_tensor_tensor) so
#      neither engine serializes the chain. h blocks DMA to DRAM in
#      SBUF-native layout (contiguous 4KB descriptors); the final
#      [b, t, hid] permutation happens host-side in numpy.

import os
import sys

sys.path.insert(0, "/opt/trn_rl_repo")

import numpy as np
import ml_dtypes

import concourse.bass as bass
import concourse.tile as tile
from concourse import bacc, mybir
from concourse.bass_utils import run_bass_kernel_spmd

F32 = mybir.dt.float32
BF16 = mybir.dt.bfloat16
BF = ml_dtypes.bfloat16

NCORES = 8
BATCH = 64
BC = BATCH // NCORES          # batch rows per core = 8
FEAT = 128
HID = 512
ORDER = 256
TFULL = 1024
TBLK = 64                     # seq-loop steps per out-DMA block

# phase-E slot schedule: (mc, kc) per PE slot, kc-major. The PE issues in
# order, so slots 1-8 (consuming h-chunks 0,1 = early act) run while the
# previous step's late activation (chunks 2,3) is still in flight; only
# one act round-trip (chunks 2,3 -> slots 9-16) sits on the serial cycle.
SLOT2MK = [(mc, kc) for kc in range(4) for mc in range(4)]

# module-level stash for test harness introspection
last_run_info = {}


def _dap(handle, offset, dims):
    """Build an explicit AP on a DRAM tensor: dims = [[step, count], ...]
    (element units; first dim pairs with the SBUF partition dim)."""
    base = handle[:]
    return bass.AP(tensor=base.tensor, offset=offset, ap=[list(d) for d in dims])


def build_nc(T=TFULL, tblk=TBLK):
    """Emit the per-core Bass/Tile program (SPMD; all cores identical)."""
    nblk = T // tblk
    BT = BC * T                       # rows of x per core
    nxt = BT // 128                   # 128-row x tiles
    th_n = T // 512 if T >= 512 else 1  # 512-wide tau halves in conv
    tw = min(T, 512)                  # conv tau tile width
    OWB = tblk * 4 * BC               # out elements per partition per block

    nc = bacc.Bacc(None, target_bir_lowering=False)
    x_d = nc.declare_dram_parameter("x", [BT, FEAT], F32, isOutput=False)
    whT_d = nc.declare_dram_parameter("whT", [HID, HID], BF16, isOutput=False)
    g_d = nc.declare_dram_parameter("g", [T, HID], BF16, isOutput=False)
    wx_d = nc.declare_dram_parameter("wx", [FEAT, HID], BF16, isOutput=False)
    ex_d = nc.declare_dram_parameter("ex", [FEAT, 1], BF16, isOutput=False)
    id_d = nc.declare_dram_parameter("ident", [128, 128], F32, isOutput=False)
    # out in SBUF-native block layout: row = blk*128 + p, col = dt*32+mc*8+b8
    out_d = nc.declare_dram_parameter("out", [nblk * 128, OWB], BF16,
                                      isOutput=True)

    UPADW = 512 + T                   # zeros(512) ++ u(T)
    upad_d = nc.dram_tensor("u_pad", [BC, UPADW], BF16)

    USHW = T + 384                    # Qi domain width
    KCN = T // 128                    # lag chunks

    with tile.TileContext(nc) as tc:
        with (
            tc.tile_pool(name="consts", bufs=1) as consts,
            tc.tile_pool(name="work", bufs=4) as work,
            tc.tile_pool(name="hout", bufs=2) as hpool,
            tc.tile_pool(name="psA", bufs=4, space="PSUM") as psA,
            tc.tile_pool(name="psS", bufs=4, space="PSUM") as psS,
        ):
            # ---- resident constants -------------------------------------
            whT_sb = consts.tile([128, 4, HID], BF16)
            for kc in range(4):
                nc.sync.dma_start(out=whT_sb[:, kc, :],
                                  in_=whT_d[kc * 128:(kc + 1) * 128, :])
            g_sb = consts.tile([128, KCN, HID], BF16)
            for kc in range(KCN):
                nc.sync.dma_start(out=g_sb[:, kc, :],
                                  in_=g_d[kc * 128:(kc + 1) * 128, :])
            wx_sb = consts.tile([128, HID], BF16)
            nc.sync.dma_start(out=wx_sb, in_=wx_d[:, :])
            ex_sb = consts.tile([128, 1], BF16)
            nc.sync.dma_start(out=ex_sb, in_=ex_d[:, :])
            id_sb = consts.tile([128, 128], F32)
            nc.sync.dma_start(out=id_sb, in_=id_d[:, :])

            xT_sb = consts.tile([128, BT], BF16)    # x.T : [feat, (b,tau)]
            ushr = consts.tile([128, BC, USHW], BF16)  # reversed u shifts
            c_sb = consts.tile([128, 4, BC, T], BF16)  # c^T resident
            zrow = consts.tile([1, 512], BF16)
            nc.vector.memset(zrow, 0.0)
            h0 = consts.tile([128, 4, BC], BF16)
            nc.vector.memset(h0, 0.0)

            # ---- phase A: x transpose (PE) ------------------------------
            for r in range(nxt):
                x_tile = work.tile([128, 128], F32, tag="xt")
                nc.sync.dma_start(out=x_tile,
                                  in_=x_d[r * 128:(r + 1) * 128, :])
                ps = psA.tile([128, 128], F32, tag="ps")
                nc.tensor.transpose(ps, x_tile, id_sb)
                dst = xT_sb[:, r * 128:(r + 1) * 128]
                if r % 2 == 0:
                    nc.scalar.copy(dst, ps)
                else:
                    nc.vector.tensor_copy(dst, ps)

            # ---- phase B: u = x @ e_x  ->  u_pad DRAM -------------------
            for b8 in range(BC):
                urow = work.tile([1, UPADW], BF16, tag="urow")
                nc.vector.tensor_copy(urow[:, 0:512], zrow)
                for th in range((T + 511) // 512):
                    w = min(512, T - th * 512)
                    ps = psA.tile([1, 512], F32, tag="ps")
                    nc.tensor.matmul(ps[:, :w], lhsT=ex_sb,
                                     rhs=xT_sb[:, b8 * T + th * 512:
                                               b8 * T + th * 512 + w],
                                     start=True, stop=True)
                    nc.scalar.copy(urow[:, 512 + th * 512:512 + th * 512 + w],
                                   ps[:, :w])
                nc.gpsimd.dma_start(out=upad_d[b8:b8 + 1, :], in_=urow)

            # ---- phase C: build reversed shift matrix -------------------
            # ushr[p, b, Qi] = u_pad[b][1 + Qi + p]
            for b8 in range(BC):
                nc.gpsimd.dma_start(
                    out=ushr[:, b8, :],
                    in_=_dap(upad_d, b8 * UPADW + 1, [[1, 128], [1, USHW]]))

            # ---- phase D: c^T = conv(G, u) + W_x^T @ x^T -> c_sb (SBUF) -
            ev = 0
            for b8 in range(BC):
                for jt in range(4):
                    for th in range(th_n):
                        ps = psA.tile([128, tw], F32, tag="ps")
                        first = True
                        kmax = min(KCN, 4 * th + tw // 128)
                        for kc in range(kmax):
                            qi0 = 384 + 512 * th - 128 * kc
                            nc.tensor.matmul(
                                ps, lhsT=g_sb[:, kc, jt * 128:(jt + 1) * 128],
                                rhs=ushr[:, b8, qi0:qi0 + tw],
                                start=first, stop=False)
                            first = False
                        nc.tensor.matmul(
                            ps, lhsT=wx_sb[:, jt * 128:(jt + 1) * 128],
                            rhs=xT_sb[:, b8 * T + th * 512:b8 * T + th * 512 + tw],
                            start=False, stop=True)
                        dst = c_sb[:, jt, b8, th * 512:th * 512 + tw]
                        if ev % 2 == 0:
                            nc.scalar.copy(dst, ps)
                        else:
                            nc.vector.tensor_copy(dst, ps)
                        ev += 1

            # ---- phase E: sequential h recurrence -----------------------
            # Warm all psS banks once: a start=True pass clears the
            # pending-zero bits over our [128, 4*BC] region so the per-step
            # matmuls can run start=False and accumulate onto a DVE/ACT
            # prewritten c_t (keeps the c add off the PE critical path).
            warm = [psS.tile([128, 4, BC], F32, tag="pss", name=f"warm{i}")
                    for i in range(4)]
            for mc in range(4):
                for wt in warm:
                    nc.tensor.matmul(
                        wt[:, mc, :],
                        lhsT=whT_sb[:, 0, mc * 128:(mc + 1) * 128],
                        rhs=h0[:, 0, :],
                        start=(mc == 0), stop=(mc == 3),
                        skip_group_check=True)

            h_prev = h0                      # [128, 4(kc), BC] bf16
            h_prev_dt = None
            ps_cur = psS.tile([128, 4, BC], F32, tag="pss")
            nc.vector.tensor_copy(ps_cur, c_sb[:, :, :, 0])
            for blk in range(nblk):
                t0 = blk * tblk
                hb = hpool.tile([128, tblk, 4, BC], BF16, tag="hb")
                for dt in range(tblk):
                    t = t0 + dt
                    ps = ps_cur
                    # prefetch next step's c into its psum bank (DVE, off
                    # the PE critical path)
                    if t + 1 < T:
                        ps_cur = psS.tile([128, 4, BC], F32, tag="pss")
                        nc.vector.tensor_copy(ps_cur, c_sb[:, :, :, t + 1])
                    for slot, (mc, kc) in enumerate(SLOT2MK):
                        rhs = (h_prev[:, kc, :] if h_prev_dt is None
                               else h_prev[:, h_prev_dt, kc, :])
                        nc.tensor.matmul(
                            ps[:, mc, :],
                            lhsT=whT_sb[:, kc, mc * 128:(mc + 1) * 128],
                            rhs=rhs,
                            start=False, stop=False,
                            skip_group_check=True)
                        if slot == 13:
                            # banks 0,1 done (slots 13,14) -> early act on
                            # DVE (leaky-relu as mul-to-SBUF then max; only
                            # one PSUM input per instruction is allowed).
                            # Completes while PE runs slots 15-16 + next
                            # step's slots 1-8, so it is off the cycle.
                            lrt = work.tile([128, 2, BC], F32, tag="lrt")
                            nc.vector.tensor_scalar_mul(
                                lrt, ps[:, 0:2, :], 0.2)
                            nc.vector.tensor_tensor(
                                hb[:, dt, 0:2, :], lrt, ps[:, 0:2, :],
                                op=mybir.AluOpType.max)
                    # banks 2,3 done (slots 15,16) -> late act, single
                    # scalar PRELU: the one act round-trip on the serial
                    # per-step cycle (feeds next step's slots 9-16).
                    nc.scalar.activation(
                        hb[:, dt, 2:4, :], ps[:, 2:4, :],
                        mybir.ActivationFunctionType.Prelu, alpha=0.2)
                    h_prev = hb
                    h_prev_dt = dt
                # write block: SBUF-native layout, contiguous per partition
                nc.sync.dma_start(
                    out=_dap(out_d, blk * 128 * OWB,
                             [[OWB, 128], [4 * BC, tblk], [BC, 4], [1, BC]]),
                    in_=hb)
    nc.compile()
    return nc


_nc_cache = {}


def _get_nc(T, tblk):
    key = (T, tblk)
    if key not in _nc_cache:
        _nc_cache[key] = build_nc(T, tblk)
    return _nc_cache[key]


def host_prep(x, A, Bv, W_x, e_x, W_h, W_m, T):
    """Host-side constant prep (float64, exact fn of constant inputs)."""
    order = A.shape[0]
    A64 = A.astype(np.float64)
    b64 = Bv[:, 0].astype(np.float64)
    Hk = np.empty((T, order))
    v = b64.copy()
    for k in range(T):
        Hk[k] = v
        v = A64 @ v
    G = (Hk @ W_m.T.astype(np.float64)).astype(np.float32)      # (T, 512)
    # reverse lag index within each 128-chunk (matches reversed u-shift rows)
    Gr = G.reshape(T // 128, 128, -1)[:, ::-1, :].reshape(T, -1)
    Gr = np.ascontiguousarray(Gr).astype(BF)
    whT = np.ascontiguousarray(W_h.T).astype(BF)
    return Gr, whT


def kernel(x, A, Bv, W_x, e_x, W_h, W_m, T=TFULL, tblk=TBLK):
    x = np.asarray(x, np.float32)
    A = np.asarray(A, np.float32)
    Bv = np.asarray(Bv, np.float32)
    W_x = np.asarray(W_x, np.float32)
    e_x = np.asarray(e_x, np.float32)
    W_h = np.asarray(W_h, np.float32)
    W_m = np.asarray(W_m, np.float32)

    Gr, whT = host_prep(x, A, Bv, W_x, e_x, W_h, W_m, T)
    ident = np.eye(128, dtype=np.float32)

    nc = _get_nc(T, tblk)
    B = x.shape[0]
    nblk = T // tblk
    in_maps = []
    for c in range(NCORES):
        xs = np.ascontiguousarray(
            x[c * BC:(c + 1) * BC, 1:T + 1, :].reshape(BC * T, FEAT))
        in_maps.append({
            "x": xs, "whT": whT, "g": Gr, "wx": W_x.astype(BF),
            "ex": e_x.astype(BF), "ident": ident,
        })
    trace = bool(int(os.environ.get("KERNEL_TRACE", "0")))
    res = run_bass_kernel_spmd(nc, in_maps, list(range(NCORES)), trace=trace)
    last_run_info.clear()
    last_run_info.update(
        exec_time_ns=res.exec_time_ns,
        mean_exec_time_ns=res.mean_exec_time_ns,
        profile_json=res.profile_json,
    )
    out = np.empty((B, T, HID), np.float32)
    for c in range(NCORES):
        o = np.asarray(res.results[c]["out"]).astype(np.float32)
        # [blk*128+p, dt*32+mc*8+b8] -> [b8, blk*tblk+dt, mc*128+p]
        o = o.reshape(nblk, 128, tblk, 4, BC)
        o = o.transpose(4, 0, 2, 3, 1).reshape(BC, T, HID)
        out[c * BC:(c + 1) * BC] = o
    return out


# revision 9
# speedup vs baseline: 9.6057x; 1.0852x over previous
# Trainium2 Bass kernel for the KerasLMU problem.
#
# Math: per time step t (T=1024 steps),
#   u_t = x_t @ e_x                       (B,1)
#   m_t = m_{t-1} @ A.T + b_row * u_t     (B,256)   -- linear recurrence
#   h_t = lrelu(x_t @ W_x + h_{t-1} @ W_h.T + m_t @ W_m.T)
#
# Reformulation: m_t = sum_k A^k b u_{t-k} (causal convolution), so
#   c_t := x_t @ W_x + m_t @ W_m.T = x_t @ W_x + sum_k G[k] u_{t-k}
# with G[k] = W_m @ (A^k b) precomputed host-side in float64. The only
# sequential device work left is h_t = lrelu(c_t + h_{t-1} @ W_h.T).
#
# Device pipeline (per core, data-parallel over batch; 8 rows/core):
#   A: PE-transpose x -> xT (bf16)          [feat, (b,tau)]
#   B: u = e_x^T @ xT -> u_pad DRAM (bf16, 512 zeros prefix)
#   C: overlapping-window DMA -> ushr (128 shifted copies of u)
#   D: c^T = conv(G,u) + W_x^T xT, bf16 matmuls, f32 PSUM, c kept
#      RESIDENT IN SBUF as bf16 [128, 4, BC, T] (no DRAM round-trip)
#   E: 1024-step recurrence. Per step 16 [128x128]x[128x8] bf16
#      matmuls in a staggered 4-bank slot schedule so each PSUM bank
#      completes ~8 slots before its h-chunk is consumed next step,
#      hiding the PE->act->PE latency (~570ns). Leaky-ReLU alternates
#      between Scalar (Prelu) and Vector (scalar_tensor_tensor) so
#      neither engine serializes the chain. h blocks DMA to DRAM in
#      SBUF-native layout (contiguous 4KB descriptors); the final
#      [b, t, hid] permutation happens host-side in numpy.

import os
import sys

sys.path.insert(0, "/opt/trn_rl_repo")

import numpy as np
import ml_dtypes

import concourse.bass as bass
import concourse.tile as tile
from concourse import bacc, mybir
from concourse.bass_utils import run_bass_kernel_spmd

F32 = mybir.dt.float32
BF16 = mybir.dt.bfloat16
BF = ml_dtypes.bfloat16

NCORES = 8
BATCH = 64
BC = BATCH // NCORES          # batch rows per core = 8
FEAT = 128
HID = 512
ORDER = 256
TFULL = 1024
TBLK = 64                     # seq-loop steps per out-DMA block

# phase-E: the per-core batch of 8 is split into two independent 4-col
# streams, interleaved on the PE. While stream A's activation round-trip
# (~400ns of sem+act latency) is in flight, the PE runs stream B's 16
# matmuls, and vice versa -- the PE never idles, so the recurrence runs at
# PE issue rate (~32 matmuls/step) instead of act-latency rate.
SLOT2MK = [(mc, kc) for kc in range(4) for mc in range(4)]
BCH = BC // 2                 # cols per stream = 4

# module-level stash for test harness introspection
last_run_info = {}


def _dap(handle, offset, dims):
    """Build an explicit AP on a DRAM tensor: dims = [[step, count], ...]
    (element units; first dim pairs with the SBUF partition dim)."""
    base = handle[:]
    return bass.AP(tensor=base.tensor, offset=offset, ap=[list(d) for d in dims])


def build_nc(T=TFULL, tblk=TBLK):
    """Emit the per-core Bass/Tile program (SPMD; all cores identical)."""
    nblk = T // tblk
    BT = BC * T                       # rows of x per core
    nxt = BT // 128                   # 128-row x tiles
    th_n = T // 512 if T >= 512 else 1  # 512-wide tau halves in conv
    tw = min(T, 512)                  # conv tau tile width
    OWB = tblk * 4 * BC               # out elements per partition per block

    nc = bacc.Bacc(None, target_bir_lowering=False)
    x_d = nc.declare_dram_parameter("x", [BT, FEAT], F32, isOutput=False)
    whT_d = nc.declare_dram_parameter("whT", [HID, HID], BF16, isOutput=False)
    g_d = nc.declare_dram_parameter("g", [T, HID], BF16, isOutput=False)
    wx_d = nc.declare_dram_parameter("wx", [FEAT, HID], BF16, isOutput=False)
    ex_d = nc.declare_dram_parameter("ex", [FEAT, 1], BF16, isOutput=False)
    id_d = nc.declare_dram_parameter("ident", [128, 128], F32, isOutput=False)
    # out in SBUF-native block layout: row = blk*128 + p, col = dt*32+mc*8+b8
    out_d = nc.declare_dram_parameter("out", [nblk * 128, OWB], BF16,
                                      isOutput=True)

    UPADW = 512 + T                   # zeros(512) ++ u(T)
    upad_d = nc.dram_tensor("u_pad", [BC, UPADW], BF16)

    USHW = T + 384                    # Qi domain width
    KCN = T // 128                    # lag chunks

    with tile.TileContext(nc) as tc:
        with (
            tc.tile_pool(name="consts", bufs=1) as consts,
            tc.tile_pool(name="work", bufs=4) as work,
            tc.tile_pool(name="hout", bufs=2) as hpool,
            tc.tile_pool(name="psA", bufs=4, space="PSUM") as psA,
            tc.tile_pool(name="psS", bufs=4, space="PSUM") as psS,
        ):
            # ---- resident constants -------------------------------------
            whT_sb = consts.tile([128, 4, HID], BF16)
            for kc in range(4):
                nc.sync.dma_start(out=whT_sb[:, kc, :],
                                  in_=whT_d[kc * 128:(kc + 1) * 128, :])
            g_sb = consts.tile([128, KCN, HID], BF16)
            for kc in range(KCN):
                nc.sync.dma_start(out=g_sb[:, kc, :],
                                  in_=g_d[kc * 128:(kc + 1) * 128, :])
            wx_sb = consts.tile([128, HID], BF16)
            nc.sync.dma_start(out=wx_sb, in_=wx_d[:, :])
            ex_sb = consts.tile([128, 1], BF16)
            nc.sync.dma_start(out=ex_sb, in_=ex_d[:, :])
            id_sb = consts.tile([128, 128], F32)
            nc.sync.dma_start(out=id_sb, in_=id_d[:, :])

            xT_sb = consts.tile([128, BT], BF16)    # x.T : [feat, (b,tau)]
            ushr = consts.tile([128, BC, USHW], BF16)  # reversed u shifts
            c_sb = consts.tile([128, 4, BC, T], BF16)  # c^T resident
            zrow = consts.tile([1, 512], BF16)
            nc.vector.memset(zrow, 0.0)
            h0 = consts.tile([128, 4, BC], BF16)
            nc.vector.memset(h0, 0.0)

            # ---- phase A: x transpose (PE) ------------------------------
            for r in range(nxt):
                x_tile = work.tile([128, 128], F32, tag="xt")
                nc.sync.dma_start(out=x_tile,
                                  in_=x_d[r * 128:(r + 1) * 128, :])
                ps = psA.tile([128, 128], F32, tag="ps")
                nc.tensor.transpose(ps, x_tile, id_sb)
                dst = xT_sb[:, r * 128:(r + 1) * 128]
                if r % 2 == 0:
                    nc.scalar.copy(dst, ps)
                else:
                    nc.vector.tensor_copy(dst, ps)

            # ---- phase B: u = x @ e_x  ->  u_pad DRAM -------------------
            for b8 in range(BC):
                urow = work.tile([1, UPADW], BF16, tag="urow")
                nc.vector.tensor_copy(urow[:, 0:512], zrow)
                for th in range((T + 511) // 512):
                    w = min(512, T - th * 512)
                    ps = psA.tile([1, 512], F32, tag="ps")
                    nc.tensor.matmul(ps[:, :w], lhsT=ex_sb,
                                     rhs=xT_sb[:, b8 * T + th * 512:
                                               b8 * T + th * 512 + w],
                                     start=True, stop=True)
                    nc.scalar.copy(urow[:, 512 + th * 512:512 + th * 512 + w],
                                   ps[:, :w])
                nc.gpsimd.dma_start(out=upad_d[b8:b8 + 1, :], in_=urow)

            # ---- phase C: build reversed shift matrix -------------------
            # ushr[p, b, Qi] = u_pad[b][1 + Qi + p]
            for b8 in range(BC):
                nc.gpsimd.dma_start(
                    out=ushr[:, b8, :],
                    in_=_dap(upad_d, b8 * UPADW + 1, [[1, 128], [1, USHW]]))

            # ---- phase D: c^T = conv(G, u) + W_x^T @ x^T -> c_sb (SBUF) -
            ev = 0
            for b8 in range(BC):
                for jt in range(4):
                    for th in range(th_n):
                        ps = psA.tile([128, tw], F32, tag="ps")
                        first = True
                        kmax = min(KCN, 4 * th + tw // 128)
                        for kc in range(kmax):
                            qi0 = 384 + 512 * th - 128 * kc
                            nc.tensor.matmul(
                                ps, lhsT=g_sb[:, kc, jt * 128:(jt + 1) * 128],
                                rhs=ushr[:, b8, qi0:qi0 + tw],
                                start=first, stop=False)
                            first = False
                        nc.tensor.matmul(
                            ps, lhsT=wx_sb[:, jt * 128:(jt + 1) * 128],
                            rhs=xT_sb[:, b8 * T + th * 512:b8 * T + th * 512 + tw],
                            start=False, stop=True)
                        dst = c_sb[:, jt, b8, th * 512:th * 512 + tw]
                        if ev % 2 == 0:
                            nc.scalar.copy(dst, ps)
                        else:
                            nc.vector.tensor_copy(dst, ps)
                        ev += 1

            # ---- phase E: sequential h recurrence -----------------------
            # Warm all psS banks once: a start=True pass clears the
            # pending-zero bits over our regions so the per-step matmuls
            # can run start=False and accumulate onto a prewritten c_t
            # (keeps the c add off the PE critical path).
            warm = [psS.tile([128, 4, BC], F32, tag="pss", name=f"warm{i}")
                    for i in range(4)]
            for mc in range(4):
                for wt in warm:
                    nc.tensor.matmul(
                        wt[:, mc, :],
                        lhsT=whT_sb[:, 0, mc * 128:(mc + 1) * 128],
                        rhs=h0[:, 0, :],
                        start=(mc == 0), stop=(mc == 3),
                        skip_group_check=True)

            # one PSUM tile per step; stream A uses cols 0:BCH, stream B
            # cols BCH:BC (disjoint regions, independent subtile deps)
            h_prev = h0                      # [128, 4(kc), BC] bf16
            h_prev_dt = None
            ps_cur = psS.tile([128, 4, BC], F32, tag="pss")
            nc.vector.tensor_copy(ps_cur[:, :, 0:BCH], c_sb[:, :, 0:BCH, 0])
            nc.scalar.copy(ps_cur[:, :, BCH:BC], c_sb[:, :, BCH:BC, 0])
            for blk in range(nblk):
                t0 = blk * tblk
                hb = hpool.tile([128, tblk, 4, BC], BF16, tag="hb")
                for dt in range(tblk):
                    t = t0 + dt
                    ps = ps_cur
                    pa, pb = ps[:, :, 0:BCH], ps[:, :, BCH:BC]
                    # prefetch step t+1's c for stream A (DVE; queued ahead
                    # of this step's B-act so it never blocks)
                    if t + 1 < T:
                        ps_cur = psS.tile([128, 4, BC], F32, tag="pss")
                        nc.vector.tensor_copy(ps_cur[:, :, 0:BCH],
                                              c_sb[:, :, 0:BCH, t + 1])
                    for mc, kc in SLOT2MK:
                        rhs = (h_prev[:, kc, 0:BCH] if h_prev_dt is None
                               else h_prev[:, h_prev_dt, kc, 0:BCH])
                        nc.tensor.matmul(
                            pa[:, mc, :],
                            lhsT=whT_sb[:, kc, mc * 128:(mc + 1) * 128],
                            rhs=rhs,
                            start=False, stop=False,
                            skip_group_check=True)
                    # stream A act: single scalar PRELU; round-trip hides
                    # under stream B's 16 matmuls below.
                    nc.scalar.activation(
                        hb[:, dt, :, 0:BCH], pa,
                        mybir.ActivationFunctionType.Prelu, alpha=0.2)
                    # prefetch step t+1's c for stream B (scalar, queued
                    # right after the PRELU; done well before B(t+1))
                    if t + 1 < T:
                        nc.scalar.copy(ps_cur[:, :, BCH:BC],
                                       c_sb[:, :, BCH:BC, t + 1])
                    for mc, kc in SLOT2MK:
                        rhs = (h_prev[:, kc, BCH:BC] if h_prev_dt is None
                               else h_prev[:, h_prev_dt, kc, BCH:BC])
                        nc.tensor.matmul(
                            pb[:, mc, :],
                            lhsT=whT_sb[:, kc, mc * 128:(mc + 1) * 128],
                            rhs=rhs,
                            start=False, stop=False,
                            skip_group_check=True)
                    # stream B act on DVE (leaky-relu as mul-to-SBUF then
                    # max; only one PSUM input per instruction); hides
                    # under stream A's matmuls of step t+1.
                    lrt = work.tile([128, 4, BCH], F32, tag="lrt")
                    nc.vector.tensor_scalar_mul(lrt, pb, 0.2)
                    nc.vector.tensor_tensor(
                        hb[:, dt, :, BCH:BC], lrt, pb,
                        op=mybir.AluOpType.max)
                    h_prev = hb
                    h_prev_dt = dt
                # write block: SBUF-native layout, contiguous per partition
                nc.sync.dma_start(
                    out=_dap(out_d, blk * 128 * OWB,
                             [[OWB, 128], [4 * BC, tblk], [BC, 4], [1, BC]]),
                    in_=hb)
    nc.compile()
    return nc


_nc_cache = {}


def _get_nc(T, tblk):
    key = (T, tblk)
    if key not in _nc_cache:
        _nc_cache[key] = build_nc(T, tblk)
    return _nc_cache[key]


def host_prep(x, A, Bv, W_x, e_x, W_h, W_m, T):
    """Host-side constant prep (float64, exact fn of constant inputs)."""
    order = A.shape[0]
    A64 = A.astype(np.float64)
    b64 = Bv[:, 0].astype(np.float64)
    Hk = np.empty((T, order))
    v = b64.copy()
    for k in range(T):
        Hk[k] = v
        v = A64 @ v
    G = (Hk @ W_m.T.astype(np.float64)).astype(np.float32)      # (T, 512)
    # reverse lag index within each 128-chunk (matches reversed u-shift rows)
    Gr = G.reshape(T // 128, 128, -1)[:, ::-1, :].reshape(T, -1)
    Gr = np.ascontiguousarray(Gr).astype(BF)
    whT = np.ascontiguousarray(W_h.T).astype(BF)
    return Gr, whT


def kernel(x, A, Bv, W_x, e_x, W_h, W_m, T=TFULL, tblk=TBLK):
    x = np.asarray(x, np.float32)
    A = np.asarray(A, np.float32)
    Bv = np.asarray(Bv, np.float32)
    W_x = np.asarray(W_x, np.float32)
    e_x = np.asarray(e_x, np.float32)
    W_h = np.asarray(W_h, np.float32)
    W_m = np.asarray(W_m, np.float32)

    Gr, whT = host_prep(x, A, Bv, W_x, e_x, W_h, W_m, T)
    ident = np.eye(128, dtype=np.float32)

    nc = _get_nc(T, tblk)
    B = x.shape[0]
    nblk = T // tblk
    in_maps = []
    for c in range(NCORES):
        xs = np.ascontiguousarray(
            x[c * BC:(c + 1) * BC, 1:T + 1, :].reshape(BC * T, FEAT))
        in_maps.append({
            "x": xs, "whT": whT, "g": Gr, "wx": W_x.astype(BF),
            "ex": e_x.astype(BF), "ident": ident,
        })
    trace = bool(int(os.environ.get("KERNEL_TRACE", "0")))
    res = run_bass_kernel_spmd(nc, in_maps, list(range(NCORES)), trace=trace)
    last_run_info.clear()
    last_run_info.update(
        exec_time_ns=res.exec_time_ns,
        mean_exec_time_ns=res.mean_exec_time_ns,
        profile_json=res.profile_json,
    )
    out = np.empty((B, T, HID), np.float32)
    for c in range(NCORES):
        o = np.asarray(res.results[c]["out"]).astype(np.float32)
        # [blk*128+p, dt*32+mc*8+b8] -> [b8, blk*tblk+dt, mc*128+p]
        o = o.reshape(nblk, 128, tblk, 4, BC)
        o = o.transpose(4, 0, 2, 3, 1).reshape(BC, T, HID)
        out[c * BC:(c + 1) * BC] = o
    return out
